# revision 80
# baseline (speedup 1.0000x reference)
"""NeighborMLPConvLayer Trainium2 kernel — zero-padding block-sum design.

Strategy (8 NeuronCores, SPMD, edge-parallel):
  - The host pre-gathers per-edge features into a dense bf16 stream
    feat[65, e] = [in_features[nbr] (32); out_features[seg] (32); 1].
    The ones-row folds b1 into the W1 matmul.  No dma_gather and no
    one-hot matrices on-device: DMA is one dense 2B/elem stream in and a
    small partial-sum stream out.
  - Edges (already sorted by destination segment) are split into region A
    (each segment's floor(d/4)*4 edges, grouped into static 4-slot blocks)
    and region B (the d%4 orphan edges, one y row each) — zero padding
    waste, which matters because the Activation engine's exact-erf Gelu
    over E*H elements is the wall this kernel sits on (~94% busy).
  - Region A, per 128-slot chunk: h_T[e,128] = feat_c.T @ [W1;b1] (PSUM),
    Gelu (PSUM->SBUF bf16), block-sum hblk[H,16] = hp_c.T @ B4 against a
    constant block-ones matrix (segment-sum becomes shape-static), then
    yblk[blocks,64] = hblk.T @ W2 per 128-block panel, reusing the dead
    hblk PSUM region.  Region B: wide M1 (h[H,e]), Gelu, chunked W2.
  - The per-group stages are software-pipelined with one group of skew so
    M1(g+1) issues on the PE ahead of group g's block-sum tail and the
    Activation engine runs gap-free; feature tiles prefetch 1-2 loads
    ahead; a warmup matmul run holds the PE at full p-state for group 0.
  - Partial sums (f32) stream to DRAM; the host finishes with a
    block->segment reduceat, divides by counts and adds b2.
"""

import sys

sys.path.insert(0, "/opt/trn_rl_repo")

import numpy as np
import ml_dtypes

BF16 = ml_dtypes.bfloat16

# Problem geometry (hardcoded per the task contract).
N = 50000
M = 50000
C = 32
H = 128
O = 64
E = 1_600_000
NCORES = 8

F = 2 * C + 1          # feature rows streamed per edge (rep, slf, ones)
B = 4                  # slots per block in region A
CH = 128               # slots per chunk (matmul partition dim)
GCH = 12               # chunks per region-A group
GRP_SLOTS = GCH * CH   # slots per region-A group
GRP_BLKS = GRP_SLOTS // B  # blocks per region-A group
PANELS = -(-GRP_BLKS // CH)  # 128-block W2 panels per group (last overlaps)
FDM = 4                # groups per feature DMA (last load may be shorter)
OD = 2                 # groups per output DMA
BCH = 8                # chunks per region-B group (orphan edges, 1 block each)
BG_SLOTS = BCH * CH    # slots per region-B group

_prog_cache = {}


# ----------------------------------------------------------------- host prep

def _host_prep(in_features, out_features, W1, b1, W2, b2,
               neighbors_index, neighbors_row_splits):
    rs = np.asarray(neighbors_row_splits).astype(np.int64)
    idx = np.asarray(neighbors_index).astype(np.int64)
    counts = np.diff(rs)                                   # [M]
    a_d = (counts // B) * B        # region-A edges per segment (no padding)
    b_d = counts - a_d             # region-B orphan edges per segment (0..B-1)
    nbA = a_d // B                 # region-A blocks per segment
    eA = int(a_d.sum())
    eB = int(b_d.sum())
    nblkA = eA // B

    # per-core slot budgets, chunk aligned (the final group of each region
    # may be a "runt" with fewer chunks; trailing slots zero-filled)
    sA = -(-eA // (NCORES * CH)) * CH
    sB = -(-eB // (NCORES * CH)) * CH
    nGA = -(-sA // GRP_SLOTS)
    nGB = -(-sB // BG_SLOTS)
    rchA = (sA - (nGA - 1) * GRP_SLOTS) // CH   # chunks in last A group
    rchB = (sB - (nGB - 1) * BG_SLOTS) // CH    # chunks in last B group

    a_start = np.zeros(M + 1, np.int64)
    np.cumsum(a_d, out=a_start[1:])
    b_start = np.zeros(M + 1, np.int64)
    np.cumsum(b_d, out=b_start[1:])
    seg_ids = np.repeat(np.arange(M, dtype=np.int64), counts)
    rank = np.arange(E, dtype=np.int64) - rs[seg_ids]
    in_a = rank < a_d[seg_ids]
    slot = np.where(in_a,
                    a_start[seg_ids] + rank,
                    NCORES * sA + b_start[seg_ids] + (rank - a_d[seg_ids]))

    # slot-major feature rows, then per-core transpose to feature-major
    feat_rows = np.zeros((NCORES * (sA + sB), F), BF16)
    feat_rows[slot, :C] = np.asarray(in_features, np.float32)[idx].astype(BF16)
    feat_rows[slot, C:2 * C] = np.asarray(out_features, np.float32)[seg_ids].astype(BF16)
    feat_rows[slot, 2 * C] = BF16(1.0)

    w1p = np.concatenate(
        [np.asarray(W1, np.float32),
         np.asarray(b1, np.float32).reshape(1, H)], axis=0).astype(BF16)
    w2 = np.asarray(W2, np.float32).astype(BF16)
    b8 = np.zeros((CH, CH // B), BF16)
    b8[np.arange(CH), np.arange(CH) // B] = BF16(1.0)

    in_maps = []
    for k in range(NCORES):
        fa = feat_rows[k * sA:(k + 1) * sA]
        fb = feat_rows[NCORES * sA + k * sB:NCORES * sA + (k + 1) * sB]
        in_maps.append(dict(
            feat=np.ascontiguousarray(np.concatenate([fa, fb], axis=0).T),
            w1p=w1p, w2=w2, b8=b8,
        ))

    meta = dict(counts=counts, a_d=a_d, b_d=b_d, nbA=nbA,
                eA=eA, eB=eB, nblkA=nblkA,
                sA=sA, sB=sB, nGA=nGA, nGB=nGB, rchA=rchA, rchB=rchB)
    return in_maps, meta


# ------------------------------------------------------------ device program

def _build_program(nGA, nGB, rchA, rchB):
    import concourse.bacc as bacc
    import concourse.mybir as mybir
    import concourse.tile as tile

    dt = mybir.dt
    nc = bacc.Bacc("TRN2", target_bir_lowering=False, debug=False)

    sA = (nGA - 1) * GRP_SLOTS + rchA * CH
    sB = (nGB - 1) * BG_SLOTS + rchB * CH
    d_feat = nc.dram_tensor("feat", [F, sA + sB], dt.bfloat16,
                            kind="ExternalInput")
    d_w1p = nc.dram_tensor("w1p", [F, H], dt.bfloat16, kind="ExternalInput")
    d_w2 = nc.dram_tensor("w2", [H, O], dt.bfloat16, kind="ExternalInput")
    d_b8 = nc.dram_tensor("b8", [CH, CH // B], dt.bfloat16, kind="ExternalInput")
    # region-A rows ordered (q, p, w) per OD-group pair so each partition's
    # rows of 64 floats are contiguous (>=512B descriptors, no sub-512B DMA
    # penalty); region-B rows (p, c) per group follow.
    d_out = nc.dram_tensor("out_blocks",
                           [nGA * PANELS * CH + nGB * BG_SLOTS, O],
                           dt.float32, kind="ExternalOutput")

    from contextlib import ExitStack

    with tile.TileContext(nc) as tc, ExitStack() as ctx:
        cpool = ctx.enter_context(tc.tile_pool(name="consts", bufs=1))
        fpool = ctx.enter_context(tc.tile_pool(name="feat", bufs=2))
        fbpool = ctx.enter_context(tc.tile_pool(name="featb", bufs=10))
        hppool = ctx.enter_context(tc.tile_pool(name="hp", bufs=2))
        ypool = ctx.enter_context(tc.tile_pool(name="yout", bufs=2))
        ybpool = ctx.enter_context(tc.tile_pool(name="youtb", bufs=6))
        bbpool = ctx.enter_context(tc.tile_pool(name="hblk", bufs=2))
        hpsum = ctx.enter_context(tc.tile_pool(name="hps", bufs=2, space="PSUM"))
        spsum = ctx.enter_context(tc.tile_pool(name="sps", bufs=2, space="PSUM"))

        w1p_sb = cpool.tile([F, H], dt.bfloat16, tag="w1p")
        w2_sb = cpool.tile([H, O], dt.bfloat16, tag="w2")
        b8_sb = cpool.tile([CH, CH // B], dt.bfloat16, tag="b8")
        warm_sb = cpool.tile([CH, 16], dt.bfloat16, tag="warm")

        NB = CH // B                       # blocks per chunk (16)
        feat_tiles = {}
        featb_tiles = {}
        hp_tiles = {}
        yblk = None

        def load_feat(f):
            nload = min(FDM, nGA - f * FDM)
            nfull = nload - 1 if f * FDM + nload == nGA and rchA < GCH else nload
            ft = fpool.tile([F, FDM, GRP_SLOTS], dt.bfloat16, tag="feat")
            feat_tiles[f] = ft
            base = f * FDM * GRP_SLOTS
            if f == 0:
                # split the very first load so group 0 starts ~2us sooner
                nc.sync.dma_start(
                    out=ft[:, 0:1, :],
                    in_=d_feat[:, 0:GRP_SLOTS]
                        .rearrange("f (g s) -> f g s", g=1),
                )
                if nfull > 1:
                    nc.sync.dma_start(
                        out=ft[:, 1:nfull, :],
                        in_=d_feat[:, GRP_SLOTS:nfull * GRP_SLOTS]
                            .rearrange("f (g s) -> f g s", g=nfull - 1),
                    )
            elif nfull > 0:
                nc.sync.dma_start(
                    out=ft[:, 0:nfull, :],
                    in_=d_feat[:, base:base + nfull * GRP_SLOTS]
                        .rearrange("f (g s) -> f g s", g=nfull),
                )
            if nfull < nload:
                rb = base + nfull * GRP_SLOTS
                nc.sync.dma_start(
                    out=ft[:, nfull:nfull + 1, 0:rchA * CH],
                    in_=d_feat[:, rb:rb + rchA * CH]
                        .rearrange("f (g s) -> f g s", g=1),
                )

        def head_a(g):
            """feat prefetch (a full tile ahead), M1 matmuls, gelu."""
            nch = rchA if g == nGA - 1 else GCH
            f, gg = divmod(g, FDM)
            if gg == 0:
                if f not in feat_tiles:
                    load_feat(f)
                if f + 1 < -(-nGA // FDM) and f + 1 not in feat_tiles:
                    load_feat(f + 1)
            feat_t = feat_tiles[f]
            h_ps = hpsum.tile([CH, GCH * H], dt.float32, tag="h")
            for c in range(nch):
                nc.tensor.matmul(
                    h_ps[:, c * H:(c + 1) * H],
                    lhsT=feat_t[:, gg, c * CH:(c + 1) * CH],
                    rhs=w1p_sb[:],
                    start=True, stop=True,
                )
            hp = hppool.tile([CH, GCH * H], dt.bfloat16, tag="hp")
            nc.scalar.activation(
                hp[:, 0:nch * H], h_ps[:, 0:nch * H],
                func=mybir.ActivationFunctionType.Gelu,
                bias=0.0, scale=1.0,
            )
            hp_tiles[("A", g)] = hp

        def tail_a(g):
            """block-sum, W2 panels, copies, out DMA for group g."""
            nonlocal yblk
            nch = rchA if g == nGA - 1 else GCH
            nblk = nch * NB
            npan = -(-nblk // CH)
            runt = nch < GCH
            hp = hp_tiles.pop(("A", g))
            s_ps = spsum.tile([CH, max(GRP_BLKS, BCH * O)], dt.float32, tag="s")
            for c in range(nch):
                nc.tensor.matmul(
                    s_ps[:, c * NB:(c + 1) * NB],
                    lhsT=hp[:, c * H:(c + 1) * H],
                    rhs=b8_sb[:],
                    start=True, stop=True,
                    skip_group_check=True,
                )
            hblk = bbpool.tile([H, GRP_BLKS], dt.bfloat16, tag="hblk")
            nc.vector.tensor_copy(out=hblk[:, 0:nblk], in_=s_ps[:, :nblk])
            # yblk[blocks, O] = hblk.T @ W2; panels reuse the (now dead)
            # leading PSUM region of s_ps after the hblk copy drained it
            pw = min(CH, nblk)
            for w in range(npan):
                p0 = w * CH if w < npan - 1 else nblk - pw
                nc.tensor.matmul(
                    s_ps[0:pw, w * O:(w + 1) * O],
                    lhsT=hblk[:, p0:p0 + pw], rhs=w2_sb[:],
                    start=True, stop=True, skip_group_check=True,
                )
            if runt:
                # runt flushes alone with its own row stride
                yr = ypool.tile([CH, OD, PANELS, O], dt.float32, tag="yblk")
                nc.vector.tensor_copy(
                    out=yr[0:pw, 0, 0:npan, :].rearrange("p w o -> p (w o)"),
                    in_=s_ps[0:pw, 0:npan * O],
                )
                nc.sync.dma_start(
                    out=d_out[g * PANELS * CH:g * PANELS * CH + npan * pw, :]
                        .rearrange("(p w) o -> p w o", w=npan),
                    in_=yr[0:pw, 0, 0:npan, :],
                )
                return
            if g % OD == 0:
                yblk = ypool.tile([CH, OD, PANELS, O], dt.float32, tag="yblk")
            nc.vector.tensor_copy(
                out=yblk[:, g % OD, :, :],
                in_=s_ps[:, 0:PANELS * O]
                    .rearrange("p (w o) -> p w o", w=PANELS),
            )
            last_full = nGA - 2 if rchA < GCH else nGA - 1
            if g % OD == OD - 1 or g == last_full:
                nq = g % OD + 1
                g0 = g - (nq - 1)
                nc.sync.dma_start(
                    out=d_out[g0 * PANELS * CH:(g + 1) * PANELS * CH, :]
                        .rearrange("(q p w) o -> p q w o", p=CH, w=PANELS),
                    in_=yblk[:, 0:nq, :, :],
                )

        def load_featb(g):
            nch = rchB if g == nGB - 1 else BCH
            ft = fbpool.tile([F, BG_SLOTS], dt.bfloat16, tag="featb")
            featb_tiles[g] = ft
            base = sA + g * BG_SLOTS
            nc.sync.dma_start(out=ft[:, 0:nch * CH],
                              in_=d_feat[:, base:base + nch * CH])

        def head_b(g):
            """region B (orphan edges): wide M1, gelu (feat prefetched)."""
            nch = rchB if g == nGB - 1 else BCH
            ns = nch * CH
            ft = featb_tiles.pop(g)[:]
            h_ps = hpsum.tile([CH, GCH * H], dt.float32, tag="h")
            for off in range(0, ns, 512):
                wd = min(512, ns - off)
                nc.tensor.matmul(h_ps[:, off:off + wd], lhsT=w1p_sb[:],
                                 rhs=ft[:, off:off + wd],
                                 start=True, stop=True, skip_group_check=True)
            hp = hppool.tile([CH, GCH * H], dt.bfloat16, tag="hp")
            nc.scalar.activation(
                hp[:, 0:ns], h_ps[:, 0:ns],
                func=mybir.ActivationFunctionType.Gelu,
                bias=0.0, scale=1.0,
            )
            hp_tiles[("B", g)] = hp

        def tail_b(g):
            """region B: per-edge y rows via chunked W2, copy, out DMA."""
            nch = rchB if g == nGB - 1 else BCH
            hp = hp_tiles.pop(("B", g))
            s_ps = spsum.tile([CH, max(GRP_BLKS, BCH * O)], dt.float32, tag="s")
            for c in range(nch):
                nc.tensor.matmul(
                    s_ps[:, c * O:(c + 1) * O],
                    lhsT=hp[:, c * CH:(c + 1) * CH], rhs=w2_sb[:],
                    start=True, stop=True, skip_group_check=True,
                )
            yb = ybpool.tile([CH, BCH, O], dt.float32, tag="yb")
            nc.vector.tensor_copy(
                out=yb[:, 0:nch, :],
                in_=s_ps[:, 0:nch * O].rearrange("p (c o) -> p c o", c=nch),
            )
            base = nGA * PANELS * CH + g * BG_SLOTS
            nc.sync.dma_start(
                out=d_out[base:base + nch * CH, :]
                    .rearrange("(p c) o -> p c o", p=CH),
                in_=yb[:, 0:nch, :],
            )

        # Prologue: w1p + the first feature tile head the SP/HWDGE queue
        # (M1(0) needs both); the Act queue stays clear so the activation
        # table load runs immediately.  A run of tiny matmuls keeps the PE
        # busy through the first DMA wait so M1(0) runs at full p-state.
        nc.sync.dma_start(out=w1p_sb[:], in_=d_w1p[:])
        load_feat(0)
        nc.sync.dma_start(out=w2_sb[:], in_=d_w2[:])
        nc.sync.dma_start(out=b8_sb[:], in_=d_b8[:])
        nc.vector.memset(warm_sb[:], 0.0)
        warm_ps = spsum.tile([CH, 16], dt.float32, tag="s")
        for _ in range(150):
            nc.tensor.matmul(warm_ps[0:16, 0:16], lhsT=warm_sb[:],
                             rhs=warm_sb[:],
                             start=True, stop=True, skip_group_check=True)

        # software pipeline with one group of skew: M1(g+1) issues on PE
        # before the block-sum tail of g, so the Activation engine never
        # waits behind PE tail work.  Region-B groups follow region A.
        seq = [("A", g) for g in range(nGA)] + [("B", g) for g in range(nGB)]
        heads = {"A": head_a, "B": head_b}
        tails = {"A": tail_a, "B": tail_b}
        # region-B feature tiles are small; trickle the loads in 8 groups
        # ahead of use so they interleave with region-A loads
        next_b = 0

        def prefetch_b(i):
            nonlocal next_b
            while next_b < nGB and i >= nGA + next_b - 8:
                load_featb(next_b)
                next_b += 1

        heads[seq[0][0]](seq[0][1])
        for i in range(1, len(seq)):
            prefetch_b(i)
            heads[seq[i][0]](seq[i][1])
            tails[seq[i - 1][0]](seq[i - 1][1])
        tails[seq[-1][0]](seq[-1][1])

    nc.compile()
    return nc


# ------------------------------------------------------------------- runner

LAST_RESULT = None


def kernel(in_features, out_features, W1, b1, W2, b2,
           neighbors_index, neighbors_row_splits):
    import os
    from concourse.bass_utils import run_bass_kernel_spmd

    in_maps, meta = _host_prep(
        in_features, out_features, W1, b1, W2, b2,
        neighbors_index, neighbors_row_splits,
    )
    nGA, nGB = meta["nGA"], meta["nGB"]
    rchA, rchB = meta["rchA"], meta["rchB"]

    key = (nGA, nGB, rchA, rchB)
    if key not in _prog_cache:
        _prog_cache[key] = _build_program(nGA, nGB, rchA, rchB)
    nc = _prog_cache[key]

    trace = bool(os.environ.get("KERNEL_TRACE"))
    if trace:
        try:
            import antenv.axon_hooks  # noqa: F401
        except ImportError:
            trace = False
    res = run_bass_kernel_spmd(nc, in_maps, core_ids=list(range(NCORES)),
                               trace=trace)
    global LAST_RESULT
    LAST_RESULT = res
    outs = res.results

    # Region-A rows per group: rows ordered (p, w) with stride = its panel
    # count; panel w holds blocks [w*CH, w*CH+CH) except the last, which
    # holds the final CH (or all, for a sub-CH runt) blocks — overlapping
    # rows are duplicates and ignored.
    def a_rows_of(nblk):
        npan = -(-nblk // CH)
        pw = min(CH, nblk)
        b_all = np.arange(nblk, dtype=np.int64)
        w_of = np.minimum(b_all // CH, npan - 1)
        last = b_all >= (npan - 1) * CH
        p_of = np.where(last, b_all - (nblk - pw), b_all % CH)
        return p_of * npan + w_of

    full_a = a_rows_of(GRP_BLKS)
    parts = [g * (PANELS * CH) + full_a for g in range(nGA - 1)]
    parts.append((nGA - 1) * (PANELS * CH) + a_rows_of(rchA * (CH // B)))
    rows_a = np.concatenate(parts)
    # Region-B rows: slot j in group g -> row g*BG + (j%CH)*nch + j//CH
    def b_rows_of(nch):
        j = np.arange(nch * CH, dtype=np.int64)
        return (j % CH) * nch + j // CH

    offB = nGA * PANELS * CH
    full_b = b_rows_of(BCH)
    parts = [offB + g * BG_SLOTS + full_b for g in range(nGB - 1)]
    parts.append(offB + (nGB - 1) * BG_SLOTS + b_rows_of(rchB))
    rows_b = np.concatenate(parts)

    nblkA = meta["nblkA"]
    eB = meta["eB"]
    blocks_a = np.empty((nblkA, O), np.float32)
    rows_b_out = np.empty((eB, O), np.float32)
    per_a = meta["sA"] // B
    per_b = meta["sB"]
    for k in range(NCORES):
        sl = np.asarray(outs[k]["out_blocks"], np.float32)
        lo, hi = k * per_a, min((k + 1) * per_a, nblkA)
        if hi > lo:
            blocks_a[lo:hi] = sl[rows_a[:hi - lo]]
        lo, hi = k * per_b, min((k + 1) * per_b, eB)
        if hi > lo:
            rows_b_out[lo:hi] = sl[rows_b[:hi - lo]]

    counts = meta["counts"]
    nbA = meta["nbA"]
    b_d = meta["b_d"]
    sums = np.zeros((M, O), np.float32)
    has_a = nbA > 0
    a_blk_start = np.zeros(M, np.int64)
    np.cumsum(nbA[:-1], out=a_blk_start[1:])
    sums[has_a] = np.add.reduceat(blocks_a, a_blk_start[has_a], axis=0)
    has_b = b_d > 0
    b_row_start = np.zeros(M, np.int64)
    np.cumsum(b_d[:-1], out=b_row_start[1:])
    sums[has_b] += np.add.reduceat(rows_b_out, b_row_start[has_b], axis=0)

    present = counts > 0
    out = np.zeros((M, O), np.float32)
    b2v = np.asarray(b2, np.float32)
    out[present] = (sums[present] / counts[present, None].astype(np.float32)
                    + b2v[None, :])
    return out
